# revision 16
# baseline (speedup 1.0000x reference)
"""Trainium2 Bass kernel for DenseKANRBF.

Computation (per reference):
    centers c_g = linspace(-1, 1, 8)  (same for every feature)
    basis[b,f,g] = exp(-(x[b,f] - c_g)^2)
    out = einsum('bfg,fgu->bu', basis, basis_kernel)
        + gelu(x @ w1 + b1, exact) @ w2 + b2 + bias

Shapes: B=1024, F=512, G=8, U=512, H=2048 (fp32).

Strategy: HYBRID sharding over the 8 NeuronCores: 4 batch shards x 2
U-shards (rb=4, ru=2).  Each core handles 256 batch rows and 256 output
columns, so per-core DMA drops from 8.6MB (pure data-parallel,
replicated weights) to ~5.8MB, while PE work stays ~17us -- balanced
against the ~16us DMA at 360GB/s.  No cross-core communication; the
host scatters shards and gathers the 4x2 output grid.

Key device-side choices (from trace/cost-model analysis of the 41us
data-parallel baseline, which was pinned at the replicated-weight DMA
floor with a 3.4us PE stall + clock droop before the KAN tail):

  - The uniform grid makes the RBF basis a geometric sequence:
        basis_g = K_g * t0 * r^g,  t0 = exp(-(x+1)^2), r = exp(4(x+1)/7)
    K_g is folded into basis_kernel on the host.  t0/r are computed on
    device (2 Exp ACTs + 2 DVE f32 ops), then 7 bf16 DVE multiplies
    produce the whole basis.  Scalar order: exp-table dummy, r, t0,
    then gelus (one implicit gelu-table load) -- MLP1 is DMA-paced so
    the late gelu start cannot stall the PSUM recycling.
  - Every dma_start costs ~640ns of serialized DIRECT2D descriptor
    generation on the issuing sequencer, so small/latency-insensitive
    loads (b1t, vecs) go on the GpSimd DGE queue while the
    PE-critical stream (xt, w1, w2, kg, out) stays in order on Sync.
  - All tensors are pre-packed/transposed on the host so every matmul
    operand is a contiguous [128, N] slice: xt/basis in f-partition
    packed layout, w1/w2/kg in contraction-partition packed layout.
  - Weight DMAs are split into 512KB chunks (w1 x4, w2 x2, kg x4) so
    matmuls track chunk arrival instead of whole-tensor completion.
  - PE order: warmup (clock ramp) -> MLP1 (64 mm) -> bias+MLP2 (34 mm)
    -> KAN (64 mm), with both batch halves accumulating in disjoint
    column ranges of ONE PSUM bank; half-0's chain stops 8 matmuls
    early so its copy+store overlaps half-1's tail.
"""

import os
from contextlib import ExitStack

import numpy as np
import ml_dtypes

import concourse.bass as bass
import concourse.bacc as bacc
import concourse.mybir as mybir
from concourse import tile
from concourse.bass_utils import run_bass_kernel_spmd

F32 = mybir.dt.float32
BF16 = mybir.dt.bfloat16
AF = mybir.ActivationFunctionType

B, F, G, U, H = 1024, 512, 8, 512, 2048
NCORES = 8
RB, RU = 4, 2  # batch shards x U shards
BL = B // RB  # 256 batch rows per core
UL = U // RU  # 256 output cols per core
NWARM = 24  # PE clock-ramp warm-up matmuls (bridge until w1 chunk 0 lands)
W1KG = [2, 2, 4, 4, 4]  # h-groups (of 4x128 f-cols each) per w1 DMA chunk
W1OFF = [0, 2, 4, 8, 12]

bf16 = ml_dtypes.bfloat16

_prog_cache = None


def _build_program():
    nc = bacc.Bacc("TRN2", target_bir_lowering=False, debug=False, num_devices=NCORES)

    # xt[p, j*256+b] = x[s*256+b, j*128+p]   (f-chunk j, batch b)
    xt_d = nc.dram_tensor("xt", [128, 4 * BL], BF16, kind="ExternalInput")
    # cols 0..15: b1 per-partition; col 16: 4/7; col 17: -1.0
    b1t_d = nc.dram_tensor("b1t", [128, 18], F32, kind="ExternalInput")
    # vecs: [0:UL]=(b2+bias) u-slice, [UL:UL+128]=ones
    vecs_d = nc.dram_tensor("vecs", [1, UL + 128], BF16, kind="ExternalInput")
    # w1 chunks by h-group k (cols k*512+kc*128+c = w1[kc*128+p, k*128+c]);
    # first chunks are small so MLP1 can start as early as possible
    w1_d = [
        nc.dram_tensor(f"w1{q}", [128, W1KG[q] * 512], BF16, kind="ExternalInput")
        for q in range(len(W1KG))
    ]
    # w2 halves: w2q[h][p, ww*256+u] = w2[(8h+ww)*128+p, uh*256+u]
    w2_d = [
        nc.dram_tensor(f"w2{q}", [128, 2048], BF16, kind="ExternalInput")
        for q in range(2)
    ]
    # kg quarters: kgq[q][p, ii*256+u] = KG[(8q+ii)*128+p, uh*256+u],
    # KG = g-major (4096, 512) basis kernel with K_g folded in
    kg_d = [
        nc.dram_tensor(f"kg{q}", [128, 2048], BF16, kind="ExternalInput")
        for q in range(4)
    ]
    out_d = nc.dram_tensor("out", [128, 2 * UL], BF16, kind="ExternalOutput")

    with ExitStack() as ctx:
        tc = ctx.enter_context(tile.TileContext(nc))
        const = ctx.enter_context(tc.tile_pool(name="const", bufs=1))
        btp = ctx.enter_context(tc.tile_pool(name="btp", bufs=7))
        htp = ctx.enter_context(tc.tile_pool(name="htp", bufs=16))
        hps_pool = ctx.enter_context(
            tc.tile_pool(name="hps", bufs=6, space=bass.MemorySpace.PSUM)
        )
        ops_pool = ctx.enter_context(
            tc.tile_pool(name="ops", bufs=1, space=bass.MemorySpace.PSUM)
        )
        oh0 = ops_pool.tile([128, UL], F32)
        oh1 = ops_pool.tile([128, UL], F32)
        oh = [oh0, oh1]

        # ---- ACT exp-table preload + PE clock warm-up (no input deps) ----
        # Warm-up matmuls accumulate garbage into oh0; the bias matmul
        # later overwrites it (start=True).  Same engine + same tile =>
        # plain program-order WAW, no cross-pool barrier, no extra bank.
        gelu_fn = AF.Identity if os.environ.get("TRN_SIM_NOGELU") else AF.Gelu
        warm = const.tile([128, 1], F32, tag="warm")
        nc.gpsimd.memset(warm[:], 0.0)
        warm2 = const.tile([128, 1], F32, tag="warm2")
        nc.scalar.activation(warm2[:], warm[:], AF.Exp)
        wl = const.tile([128, 128], BF16, tag="wl")
        nc.vector.memset(wl[:], 0.0)
        wr = const.tile([128, 256], BF16, tag="wr")
        nc.vector.memset(wr[:], 0.0)
        for _ in range(NWARM):
            nc.tensor.matmul(
                oh0[:], wl[:], wr[:], start=True, stop=True, skip_group_check=True
            )

        # ---- loads: PE-critical stream on sync, small stuff on gpsimd ----
        xt_sb = const.tile([128, 4 * BL], BF16, tag="xt")
        nc.scalar.dma_start(xt_sb[:], xt_d[:])
        b1t_sb = const.tile([128, 18], F32, tag="b1t")
        nc.gpsimd.dma_start(b1t_sb[:], b1t_d[:])
        vecs_sb = const.tile([1, UL + 128], BF16, tag="vecs")
        nc.gpsimd.dma_start(vecs_sb[:], vecs_d[:])
        w1_sb = []
        for q in range(len(W1KG)):
            t = const.tile([128, W1KG[q] * 512], BF16, tag=f"w1{q}")
            nc.sync.dma_start(t[:], w1_d[q][:])
            w1_sb.append(t)
        w2_sb = []
        for q in range(2):
            t = const.tile([128, 2048], BF16, tag=f"w2{q}")
            nc.sync.dma_start(t[:], w2_d[q][:])
            w2_sb.append(t)
        kg_sb = []
        for q in range(4):
            t = const.tile([128, 2048], BF16, tag=f"kg{q}")
            nc.sync.dma_start(t[:], kg_d[q][:])
            kg_sb.append(t)

        def w1_blk(kc, k):  # lhsT [128 f, 128 h] for f-chunk kc, h-chunk k
            q = max(i for i in range(len(W1OFF)) if W1OFF[i] <= k)
            kk = k - W1OFF[q]
            return w1_sb[q][:, kk * 512 + kc * 128 : kk * 512 + (kc + 1) * 128]

        def w2_chunk(k):  # rhs [128 h, UL] for h-chunk k
            q, ww = divmod(k, 8)
            return w2_sb[q][:, ww * UL : (ww + 1) * UL]

        def kg_chunk(i):  # rhs [128 fg, UL] for contraction chunk i of 32
            q, ii = divmod(i, 8)
            return kg_sb[q][:, ii * UL : (ii + 1) * UL]

        bcv = vecs_sb[0:1, 0:UL]
        ones = vecs_sb[0:1, UL : UL + 128]

        # ---- basis: bt[g] = t0 * r^g ----
        # r = exp((4/7)x + 4/7) straight from xt; t0 = exp(-(x+1)^2) via
        # y = x+1, s = y*y on DVE.  Both Exps write bf16; the recurrence
        # is 7 bf16 DVE multiplies.
        r_sb = const.tile([128, 4 * BL], BF16, tag="r")
        c47 = b1t_sb[:, 16:17]
        cm1 = b1t_sb[:, 17:18]
        nc.scalar.activation(r_sb[:], xt_sb[:], AF.Exp, bias=c47, scale=c47)
        y_sb = const.tile([128, 4 * BL], F32, tag="y")
        nc.vector.tensor_scalar_add(y_sb[:], xt_sb[:], 1.0)
        s_sb = const.tile([128, 4 * BL], F32, tag="s")
        nc.vector.tensor_mul(s_sb[:], y_sb[:], y_sb[:])
        t0_sb = const.tile([128, 4 * BL], BF16, tag="t0")
        nc.scalar.activation(t0_sb[:], s_sb[:], AF.Exp, scale=cm1)
        bt = [t0_sb[:]]
        for g in range(1, G):
            c = btp.tile([128, 4 * BL], BF16, tag="bt")
            nc.vector.tensor_mul(c[:], bt[-1][:], r_sb[:])
            bt.append(c[:])

        # bias matmuls ride the warm-up tail: they only need vecs (early,
        # gpsimd DGE), so they run before the w1-gated MLP1 stream starts
        for h in range(2):
            nc.tensor.matmul(
                oh[h][:], ones, bcv, start=True, stop=False, skip_group_check=True
            )

        # ---- MLP1 weight-stationary: hT psum tiles + fused-bias gelu ----
        ht = []
        for k in range(16):
            hps = hps_pool.tile([128, BL], F32)
            for kc in range(4):
                nc.tensor.matmul(
                    hps[:],
                    w1_blk(kc, k),
                    xt_sb[:, kc * BL : (kc + 1) * BL],
                    start=(kc == 0),
                    stop=(kc == 3),
                )
            t = htp.tile([128, BL], BF16, tag="ht")
            nc.scalar.activation(t[:], hps[:], gelu_fn, bias=b1t_sb[:, k : k + 1])
            ht.append(t)

        # ---- accumulation banks: MLP2 -> KAN continue the groups the
        # bias matmuls opened at the end of the warm-up block ----
        for k in range(16):
            for h in range(2):
                nc.tensor.matmul(
                    oh[h][:],
                    ht[k][:, h * 128 : (h + 1) * 128],
                    w2_chunk(k),
                    start=False,
                    stop=False,
                    skip_group_check=True,
                )
        out_sb = const.tile([128, 2 * UL], BF16, tag="outsb")

        def kan_mm(i, h, stop):
            g, j = divmod(i, 4)
            nc.tensor.matmul(
                oh[h][:],
                bt[g][:, j * BL + h * 128 : j * BL + (h + 1) * 128],
                kg_chunk(i),
                start=False,
                stop=stop,
                skip_group_check=True,
            )

        for i in range(24):
            kan_mm(i, 0, False)
            kan_mm(i, 1, False)
        for i in range(24, 32):  # half-0 finishes first -> early store
            kan_mm(i, 0, i == 31)
        nc.scalar.activation(out_sb[:, 0:UL], oh[0][:], AF.Copy)
        nc.scalar.dma_start(out_d[:, 0:UL], out_sb[:, 0:UL])
        for i in range(24, 32):
            kan_mm(i, 1, i == 31)
        nc.scalar.activation(out_sb[:, UL : 2 * UL], oh[1][:], AF.Copy)
        nc.scalar.dma_start(out_d[:, UL : 2 * UL], out_sb[:, UL : 2 * UL])

    nc.compile()
    return nc


def _host_prep(basis_kernel, mlp_w1, mlp_b1, mlp_w2, mlp_b2, bias):
    """Packing shared across cores (weights)."""
    # w1 quarters: (kc, p, k, c) -> (p, k, kc, c)
    w1r = mlp_w1.reshape(4, 128, 16, 128).transpose(1, 2, 0, 3).reshape(128, 16 * 512)
    w1q = [np.ascontiguousarray(
               w1r[:, W1OFF[q] * 512 : (W1OFF[q] + W1KG[q]) * 512]).astype(bf16)
           for q in range(len(W1KG))]
    # g-major with K_g = exp(-(2g/7)^2) folded in
    gidx = np.arange(G, dtype=np.float64)
    kscale = np.exp(-((2.0 * gidx / 7.0) ** 2)).astype(np.float32)
    KG = (basis_kernel.transpose(1, 0, 2) * kscale[:, None, None]).reshape(G * F, U)
    b1t = np.zeros((128, 18), np.float32)
    b1t[:, :16] = mlp_b1.reshape(16, 128).T
    b1t[:, 16] = 4.0 / 7.0
    b1t[:, 17] = -1.0
    return w1q, KG, b1t


def kernel(x, basis_kernel, mlp_w1, mlp_b1, mlp_w2, mlp_b2, bias):
    global _prog_cache
    x = np.asarray(x, dtype=np.float32)
    basis_kernel = np.asarray(basis_kernel, dtype=np.float32)
    mlp_w1 = np.asarray(mlp_w1, dtype=np.float32)
    mlp_b1 = np.asarray(mlp_b1, dtype=np.float32)
    mlp_w2 = np.asarray(mlp_w2, dtype=np.float32)
    mlp_b2 = np.asarray(mlp_b2, dtype=np.float32)
    bias = np.asarray(bias, dtype=np.float32)

    w1q, KG, b1t = _host_prep(basis_kernel, mlp_w1, mlp_b1, mlp_w2, mlp_b2, bias)

    # per-U-shard packs
    ush = []
    for uh in range(RU):
        KGu = KG[:, uh * UL : (uh + 1) * UL].reshape(32, 128, UL)
        kgq = [np.ascontiguousarray(
                   KGu[8 * q : 8 * q + 8].transpose(1, 0, 2).reshape(128, 8 * UL)
               ).astype(bf16) for q in range(4)]
        w2u = mlp_w2[:, uh * UL : (uh + 1) * UL].reshape(16, 128, UL)
        w2q = [np.ascontiguousarray(
                   w2u[8 * q : 8 * q + 8].transpose(1, 0, 2).reshape(128, 8 * UL)
               ).astype(bf16) for q in range(2)]
        vecs = np.zeros((1, UL + 128), bf16)
        vecs[0, :UL] = (mlp_b2 + bias)[uh * UL : (uh + 1) * UL].astype(bf16)
        vecs[0, UL:] = np.ones(128, bf16)
        ush.append((kgq, w2q, vecs))

    # per-batch-shard packs
    bsh = []
    for s in range(RB):
        xs = x[s * BL : (s + 1) * BL]  # [256, 512]
        # [256,512] -> [128, 1024]: [p, j*256+b] = xs[b, j*128+p]
        xt = np.ascontiguousarray(
            xs.reshape(BL, 4, 128).transpose(2, 1, 0).reshape(128, 4 * BL)
        ).astype(bf16)
        bsh.append(xt)

    in_maps = []
    for c in range(NCORES):
        s, uh = divmod(c, RU)
        xt = bsh[s]
        kgq, w2q, vecs = ush[uh]
        m = {"xt": xt, "b1t": b1t, "vecs": vecs}
        for q in range(len(W1KG)):
            m[f"w1{q}"] = w1q[q]
        for q in range(4):
            m[f"kg{q}"] = kgq[q]
        for q in range(2):
            m[f"w2{q}"] = w2q[q]
        in_maps.append(m)

    if _prog_cache is None:
        _prog_cache = _build_program()
    nc = _prog_cache

    trace = bool(int(os.environ.get("TRN_KERNEL_TRACE", "0")))
    if trace:
        _install_profile_hook()
    res = run_bass_kernel_spmd(
        nc,
        in_maps,
        core_ids=list(range(NCORES)),
        trace=trace,
    )
    if trace:
        print(f"HW exec time: {res.exec_time_ns} ns")
        kernel.last_results = res

    out = np.empty((B, U), np.float32)
    for c in range(NCORES):
        s, uh = divmod(c, RU)
        oc = np.asarray(res.results[c]["out"]).astype(np.float32)  # [128, 512]
        blk = oc.reshape(128, 2, UL).transpose(1, 0, 2).reshape(BL, UL)
        out[s * BL : (s + 1) * BL, uh * UL : (uh + 1) * UL] = blk
    return out.astype(np.float32)


kernel.last_results = None


def _install_profile_hook():
    """The image lacks antenv.axon_hooks; synthesize it so
    run_bass_kernel_spmd(trace=True) can reach the NTFF profiler in
    libaxon_pjrt.so.  Test-only path (TRN_KERNEL_TRACE=1)."""
    import sys
    import types

    if "antenv.axon_hooks" not in sys.modules:
        mod = types.ModuleType("antenv.axon_hooks")
        mod._hook = None

        def set_axon_ntff_profile_hook(h):
            mod._hook = h

        def get_axon_ntff_profile_hook():
            return mod._hook

        mod.set_axon_ntff_profile_hook = set_axon_ntff_profile_hook
        mod.get_axon_ntff_profile_hook = get_axon_ntff_profile_hook
        sys.modules["antenv.axon_hooks"] = mod
        import antenv

        antenv.axon_hooks = mod
        from trn_agent_boot.trn_boot import _ntff_profile_via_ctypes

        mod.set_axon_ntff_profile_hook(
            _ntff_profile_via_ctypes("/opt/axon/libaxon_pjrt.so")
        )
    import concourse.bass_utils as _bu

    _bu.upload_artifacts = lambda tmpdir: f"local:{tmpdir}"
